# revision 47
# baseline (speedup 1.0000x reference)
"""Trainium2 Bass kernel for CompositionalPINN forward.

Reference semantics (B=262144, H=256, N_STEPS=8):
    state = state_dz[:, :4]; qop = state_dz[:, 4:5]; dz_sub = state_dz[:, 5:6]/8
    n_full = floor(z_frac*8); frac = z_frac*8 - n_full
    for step in range(8):
        state += (n_full > step) * MLP(state, qop, dz_sub)        # residual MLP
    state += (frac > 1e-6) * MLP(state, qop, frac*dz_sub)
    MLP(x) = silu(silu(silu(x@W1+b1)@W2+b2)@W3+b3)@W4+b4  (6->256->256->256->4)

Strategy: pure data parallel over 8 cores.  Host transposes inputs to a
feature-major layout, precomputes the per-sample step masks, and sorts
samples by n_full (descending, dealt round-robin across cores) so each
512-sample tile only runs max(n_full)+1 MLP evals instead of 9.  The
per-tile eval schedule is baked into the compiled program (derived from
the input data; recompiles if the schedule signature changes, with an
in-process cache).

Device pipelining: each 4-tile group runs as two half-groups A/B half
a round out of phase, with per-layer stage interleaving.  B's matmuls
fill the PE queue while A's silus run on the Scalar engine and vice
versa, and each half-group's L4 + state update is deferred into the
opposite phase (and across group boundaries, draining once at program
end), so the in-order PE rarely head-blocks on a just-issued silu.
This matters doubly on TRN2: the PE only reaches its full 2.4 GHz
clock after ~3us of uninterrupted work (1.2 GHz otherwise), so every
stall also downclocks the subsequent matmul streak.  PSUM is fully
used as 4 single-buffer [128,1024] pools (8 banks): h1/h2 rotate
write-over-read in one pool pair (the rotation deps coincide with the
true pipeline deps), h3 lives in the other pair until the deferred L4
accumulates the [4,512] delta into its subregion after silu3 consumed
it — no extra banks, no false dependencies.  2161us -> 1129us vs the
eval-contiguous baseline.

Per eval (tile of 512 samples, feature-major):
  - DVE copy xm[0:8, tile] -> x16r (float32r rounding for the PE; the
    BIR verifier requires f32r matmul operands from a rounding op)
  - L1: 2 matmuls  K=8 (state,qop,dz[,dz_partial]) -> psum [128,1024]
  - silu on ACT over both psum banks in one op -> sbuf float32r
  - L2/L3: 4 matmuls each (K=128 x2 accumulate, M=128 x2)
  - L4: 2 matmuls -> delta in h3-psum[0:4, 0:512]
  - masked evals: DVE mul with a host-precomputed 4-row replicated
    mask slot; plain evals skip all mask work
  - DVE add of delta into the fp32 state rows

float32r runs the PE at full rate (1 cycle/row vs 4 for fp32; N>=256
required); state accumulation stays fp32 in SBUF.  SILU on the Scalar
engine is ~1us per [128,1024] (1 col/cycle at 1.2 GHz, dtype
independent) — 3 silus x ~288 evals ~= 870us is the scalar-engine
floor for this schedule and the binding roofline together with the
PE's ~800us of full-clock matmul work.
"""

import numpy as np
from contextlib import ExitStack

import concourse.bass as bass
import concourse.tile as tile
from concourse import bacc, mybir
from concourse.bass_utils import run_bass_kernel_spmd

F32 = mybir.dt.float32
F32R = mybir.dt.float32r
Silu = mybir.ActivationFunctionType.Silu

NCORES = 8
NTILE = 512
CHUNK_TILES = 4                     # tiles per DMA chunk
H = 256
NSTEPS = 8

# xm row layout (row 7 is a zero spare; its L1 weight rows are zero)
R_STATE = 0          # rows 0-3
R_QOP = 4
R_DZSUB = 5
R_DZPART = 6


GROUP = 4                           # tiles pipelined per round


def _round_iter(schedule, tiles):
    """Yield rounds: lists of (tile_index, eval_desc) live this round.
    Used by both the program builder and the host maskcat packer — the
    (round, tile) iteration order must stay identical."""
    chunks = [(c0, min(c0 + CHUNK_TILES, tiles))
              for c0 in range(0, tiles, CHUNK_TILES)]
    for (c0, c1) in chunks:
        for g0 in range(c0, c1, GROUP):
            group = list(range(g0, min(g0 + GROUP, c1)))
            maxev = max((len(schedule[t]) for t in group), default=0)
            for r in range(maxev):
                yield [(t, schedule[t][r]) for t in group
                       if r < len(schedule[t])]


def _emit_order(schedule, tiles):
    for rnd in _round_iter(schedule, tiles):
        yield from rnd


_BUILD_CACHE = {}

LAST_EXEC_NS = None  # set when BASSK_TRACE=1


def _install_ntff_hook():
    """The agent image lacks antenv.axon_hooks; synthesize it so
    run_bass_kernel_spmd(trace=True) can reach the NTFF profiler."""
    import sys
    import types
    if "antenv.axon_hooks" in sys.modules:
        return True
    try:
        import antenv
        from trn_agent_boot.trn_boot import _ntff_profile_via_ctypes
        hook = _ntff_profile_via_ctypes("/opt/axon/libaxon_pjrt.so")
        if hook is None:
            return False
        mod = types.ModuleType("antenv.axon_hooks")
        mod.get_axon_ntff_profile_hook = lambda: hook
        mod.set_axon_ntff_profile_hook = lambda h: None
        sys.modules["antenv.axon_hooks"] = mod
        antenv.axon_hooks = mod
        return True
    except Exception:
        return False


def _build(schedule, use_bias, n_core):
    """schedule: tuple over tiles of tuples of (is_partial, use_mask).

    Masked evals read consecutive NTILE-wide slots of the packed
    per-core mask tensor, in schedule order."""
    tiles = n_core // NTILE
    n_masked = sum(1 for tev in schedule for (_, m) in tev if m)
    nc = bacc.Bacc("TRN2", target_bir_lowering=False, debug=False,
                   num_devices=NCORES)

    xm_d = nc.declare_dram_parameter("xm", [8, n_core], F32, isOutput=False)
    mk_d = nc.declare_dram_parameter("maskcat", [4, max(1, n_masked) * NTILE],
                                     F32, isOutput=False)
    w1_d = nc.declare_dram_parameter("w1", [8, 512], F32, isOutput=False)
    w2_d = nc.declare_dram_parameter("w2", [128, 512], F32, isOutput=False)
    w3_d = nc.declare_dram_parameter("w3", [128, 512], F32, isOutput=False)
    w4_d = nc.declare_dram_parameter("w4", [128, 8], F32, isOutput=False)
    if use_bias:
        b123_d = nc.declare_dram_parameter("b123", [128, 6], F32, isOutput=False)
        b4_d = nc.declare_dram_parameter("b4r", [4, 1], F32, isOutput=False)
    out_d = nc.declare_dram_parameter("outT", [4, n_core], F32, isOutput=True)

    chunks = [(c0, min(c0 + CHUNK_TILES, tiles))
              for c0 in range(0, tiles, CHUNK_TILES)]

    with tile.TileContext(nc) as tc, ExitStack() as ctx:
        const = ctx.enter_context(tc.tile_pool(name="const", bufs=1))
        data = ctx.enter_context(tc.tile_pool(name="data", bufs=1))
        acts = ctx.enter_context(tc.tile_pool(name="acts", bufs=10))
        xr = ctx.enter_context(tc.tile_pool(name="xr", bufs=4))
        tmp = ctx.enter_context(tc.tile_pool(name="tmp", bufs=2))
        # One single-buffer psum pool per (stage position, role), created
        # interleaved so a stage-pair's two h tiles sit 4 banks apart —
        # h1/h2 rotate in pA/pB (write-over-read is the true pipeline
        # dep), h3 tiles live longer (until the deferred L4+add of the
        # opposite phase) in qA/qB.
        pA = ctx.enter_context(tc.tile_pool(name="pA", bufs=1, space="PSUM"))
        qA = ctx.enter_context(tc.tile_pool(name="qA", bufs=1, space="PSUM"))
        pB = ctx.enter_context(tc.tile_pool(name="pB", bufs=1, space="PSUM"))
        qB = ctx.enter_context(tc.tile_pool(name="qB", bufs=1, space="PSUM"))
        ps12 = [pA, pB]
        ps3 = [qA, qB]

        # ---- weights: DMA fp32 staging, DVE round to float32r
        w1_s = const.tile([8, 512], F32)
        nc.gpsimd.dma_start(out=w1_s, in_=w1_d[:, :])
        w1 = const.tile([8, 512], F32R)
        nc.gpsimd.tensor_copy(w1, w1_s)
        w2_s = const.tile([128, 512], F32)
        nc.gpsimd.dma_start(out=w2_s, in_=w2_d[:, :])
        w2 = const.tile([128, 512], F32R)
        nc.gpsimd.tensor_copy(w2, w2_s)
        w3_s = const.tile([128, 512], F32)
        nc.gpsimd.dma_start(out=w3_s, in_=w3_d[:, :])
        w3 = const.tile([128, 512], F32R)
        nc.gpsimd.tensor_copy(w3, w3_s)
        w4_s = const.tile([128, 8], F32)
        nc.gpsimd.dma_start(out=w4_s, in_=w4_d[:, :])
        w4 = const.tile([128, 8], F32R)
        nc.gpsimd.tensor_copy(w4, w4_s)
        if use_bias:
            b123 = const.tile([128, 6], F32)
            nc.gpsimd.dma_start(out=b123, in_=b123_d[:, :])
            b4r = const.tile([4, 1], F32)
            nc.gpsimd.dma_start(out=b4r, in_=b4_d[:, :])

        # ---- the full per-core dataset stays resident in SBUF
        xm = data.tile([8, n_core], F32)
        mkc = data.tile([4, max(1, n_masked) * NTILE], F32)
        nc.gpsimd.dma_start(out=mkc, in_=mk_d[:, :])
        mask_slot = [0]
        for (c0, c1) in chunks:
            nc.sync.dma_start(out=xm[:, c0 * NTILE:c1 * NTILE],
                              in_=xm_d[:, c0 * NTILE:c1 * NTILE])

        # ---- emission helpers over lists of (t, ts, isp, um).
        xs, h1ps, h1ss, h2ps, h2ss, h3ps, h3ss = {}, {}, {}, {}, {}, {}, {}

        def e_cast(live):
            # f32r input snapshot (the BIR verifier requires f32r matmul
            # operands to come from a rounding producer)
            for (t, ts, isp, um) in live:
                x16r = xr.tile([8, NTILE], F32R, tag="x16")
                nc.vector.tensor_copy(x16r, xm[:, ts])
                xs[t] = x16r

        def e_l1(live):
            for i, (t, ts, isp, um) in enumerate(live):
                w1off = 256 if isp else 0
                h1p = ps12[i % 2].tile([128, 2 * NTILE], F32, tag="h")
                nc.tensor.matmul(h1p[:, 0:NTILE], w1[:, w1off:w1off + 128],
                                 xs[t][0:8, :], start=True, stop=True)
                nc.tensor.matmul(h1p[:, NTILE:2 * NTILE],
                                 w1[:, w1off + 128:w1off + 256],
                                 xs[t][0:8, :], start=True, stop=True)
                if use_bias:
                    nc.vector.tensor_scalar_add(h1p[:, 0:NTILE], h1p[:, 0:NTILE], b123[:, 0:1])
                    nc.vector.tensor_scalar_add(h1p[:, NTILE:], h1p[:, NTILE:], b123[:, 1:2])
                h1ps[t] = h1p

        def e_silu(live, src, dst):
            for (t, ts, isp, um) in live:
                hs = acts.tile([128, 2 * NTILE], F32R, tag="h")
                nc.scalar.activation(hs, src[t], Silu)
                dst[t] = hs

        def e_l23(live, w, src, dst, pool, boff):
            for i, (t, ts, isp, um) in enumerate(live):
                hp = pool[i % 2].tile([128, 2 * NTILE], F32, tag="h")
                for mt in range(2):
                    for kt in range(2):
                        nc.tensor.matmul(
                            hp[:, mt * NTILE:(mt + 1) * NTILE],
                            w[:, kt * 256 + mt * 128: kt * 256 + (mt + 1) * 128],
                            src[t][:, kt * NTILE:(kt + 1) * NTILE],
                            start=(kt == 0), stop=(kt == 1))
                if use_bias:
                    nc.vector.tensor_scalar_add(hp[:, 0:NTILE], hp[:, 0:NTILE], b123[:, boff:boff + 1])
                    nc.vector.tensor_scalar_add(hp[:, NTILE:], hp[:, NTILE:], b123[:, boff + 1:boff + 2])
                dst[t] = hp

        def e_l4_add(live):
            # L4 into a subregion of the (consumed) h3 psum tile, then the
            # DVE state update.  Masked evals read a host-precomputed 4-row
            # replicated mask slot — plain DVE mul, no broadcast matmul.
            for (t, ts, isp, um) in live:
                d = h3ps[t][0:4, 0:NTILE]
                nc.tensor.matmul(d, w4[:, 0:4], h3ss[t][:, 0:NTILE],
                                 start=True, stop=False)
                nc.tensor.matmul(d, w4[:, 4:8], h3ss[t][:, NTILE:2 * NTILE],
                                 start=False, stop=True)
            for (t, ts, isp, um) in live:
                d = h3ps[t][0:4, 0:NTILE]
                if use_bias:
                    nc.vector.tensor_scalar_add(d, d, b4r[:, 0:1])
                if not um:
                    nc.vector.tensor_add(xm[0:4, ts], xm[0:4, ts], d)
                else:
                    j = mask_slot[0]
                    mask_slot[0] += 1
                    dm = tmp.tile([4, NTILE], F32, tag="dm")
                    nc.vector.tensor_mul(dm, d, mkc[:, j * NTILE:(j + 1) * NTILE])
                    nc.vector.tensor_add(xm[0:4, ts], xm[0:4, ts], dm)

        # ---- two half-groups A/B per 4-tile group run half a round out of
        # phase: B's matmuls keep the PE busy while A's tail silus run, and
        # each half-group's L4+state-update is deferred into the opposite
        # phase so the PE never head-blocks on a just-issued silu.  The
        # flat masked-eval order stays A(0),B(0),A(1),B(1),... — identical
        # to _emit_order.
        def mklive(ts_, r):
            return [(t, bass.ds(t * NTILE, NTILE), *schedule[t][r])
                    for t in ts_ if r < len(schedule[t])]

        # prevB carries ACROSS groups and chunks: the final half-group's
        # deferred L4+add lands inside the next group's first window, so
        # group boundaries never head-block the PE on a tail silu.
        # (Output DMA stays at program end: emitting it mid-stream was
        # measured to serialize the pipeline, +220us.)
        prevB = []
        for (c0, c1) in chunks:
            for g0 in range(c0, c1, GROUP):
                gt = list(range(g0, min(g0 + GROUP, c1)))
                half = (len(gt) + 1) // 2
                A, B = gt[:half], gt[half:]
                R = max((len(schedule[t]) for t in gt), default=0)

                for r in range(R):
                    liveA = mklive(A, r)
                    liveB = mklive(B, r)
                    e_cast(liveA)                              # w1
                    e_l1(liveA)                                # w2
                    e_silu(liveA, h1ps, h1ss)                  # w3
                    e_l23(liveA, w2, h1ss, h2ps, ps12, 2)      # w4
                    e_l4_add(prevB)                            # w5
                    e_cast(liveB)
                    e_silu(liveA, h2ps, h2ss)                  # w6
                    e_l23(liveA, w3, h2ss, h3ps, ps3, 4)       # w7
                    e_silu(liveA, h3ps, h3ss)                  # w8
                    e_l1(liveB)                                # w9
                    e_silu(liveB, h1ps, h1ss)                  # w10
                    e_l23(liveB, w2, h1ss, h2ps, ps12, 2)      # w11
                    e_l4_add(liveA)                            # w12
                    e_silu(liveB, h2ps, h2ss)                  # w13
                    e_l23(liveB, w3, h2ss, h3ps, ps3, 4)       # w14
                    e_silu(liveB, h3ps, h3ss)                  # w15
                    prevB = liveB
        e_l4_add(prevB)                                        # program tail
        # drain DMA in 8 large transfers regardless of the (finer) input
        # chunking — fewer serial sync-queue triggers in the tail
        ostep = max(CHUNK_TILES, tiles // 8)
        for c0 in range(0, tiles, ostep):
            c1 = min(c0 + ostep, tiles)
            nc.sync.dma_start(out=out_d[:, c0 * NTILE:c1 * NTILE],
                              in_=xm[0:4, c0 * NTILE:c1 * NTILE])

    nc.compile()
    return nc


def kernel(state_dz, z_frac, W1, b1, W2, b2, W3, b3, W4, b4):
    global LAST_EXEC_NS
    import os

    state_dz = np.ascontiguousarray(state_dz, dtype=np.float32)
    z_frac = np.ascontiguousarray(z_frac, dtype=np.float32)
    W1 = np.asarray(W1, np.float32); W2 = np.asarray(W2, np.float32)
    W3 = np.asarray(W3, np.float32); W4 = np.asarray(W4, np.float32)
    b1 = np.asarray(b1, np.float32); b2 = np.asarray(b2, np.float32)
    b3 = np.asarray(b3, np.float32); b4 = np.asarray(b4, np.float32)

    B = state_dz.shape[0]
    assert B % (NCORES * NTILE) == 0, f"B={B} must be divisible by {NCORES * NTILE}"
    n_core = B // NCORES
    tiles = n_core // NTILE

    # ---- host-side derived quantities (bitwise-identical fp32 ops vs jax)
    dz_sub = (state_dz[:, 5] / np.float32(8.0)).astype(np.float32)
    cont = (z_frac * np.float32(NSTEPS)).astype(np.float32)
    n_full = np.floor(cont).astype(np.float32)
    frac = (cont - n_full).astype(np.float32)
    dz_part = (frac * dz_sub).astype(np.float32)
    has_part = (frac > np.float32(1e-6)).astype(np.float32)
    n_int = np.minimum(n_full, NSTEPS).astype(np.int64)

    # ---- sort desc by n_full, deal round-robin to cores
    order = np.argsort(-n_int, kind="stable")
    perms = [order[c::NCORES] for c in range(NCORES)]

    # ---- build per-core xm arrays  [16, N_CORE]
    xms = []
    for c in range(NCORES):
        p = perms[c]
        xm = np.zeros((8, n_core), np.float32)
        xm[0:4] = state_dz[p, 0:4].T
        xm[R_QOP] = state_dz[p, 4]
        xm[R_DZSUB] = dz_sub[p]
        xm[R_DZPART] = dz_part[p]
        xms.append(xm)

    # ---- union schedule across cores (SPMD: one program for all cores)
    sched = []
    for t in range(tiles):
        sl = slice(t * NTILE, (t + 1) * NTILE)
        smax, smin = 0, NSTEPS
        anyp, allp = False, True
        for c in range(NCORES):
            nf = n_int[perms[c][sl]]
            smax = max(smax, int(nf.max()))
            smin = min(smin, int(nf.min()))
            hp = has_part[perms[c][sl]]
            anyp = anyp or bool(hp.any())
            allp = allp and bool(hp.all())
        evals = []
        steps = 0
        for s in range(min(smax, NSTEPS)):
            evals.append((False, smin <= s))
            steps += 1
        if anyp:
            evals.append((True, not allp))
        sched.append(tuple(evals))
    sched = tuple(sched)

    # masked evals in DEVICE EMISSION order; track each tile's step counter
    masked_evals = []
    step_no = [0] * tiles
    for t, (is_partial, use_mask) in _emit_order(sched, tiles):
        s = None if is_partial else step_no[t]
        if not is_partial:
            step_no[t] += 1
        if use_mask:
            masked_evals.append((t, s))

    # ---- packed mask rows (replicated to 4 partitions), one NTILE slot
    # per masked eval, per core
    nm = max(1, len(masked_evals))
    maskcats = [np.zeros((4, nm * NTILE), np.float32) for _ in range(NCORES)]
    for j, (t, s) in enumerate(masked_evals):
        sl = slice(t * NTILE, (t + 1) * NTILE)
        for c in range(NCORES):
            idx = perms[c][sl]
            row = has_part[idx] if s is None else (n_full[idx] > s).astype(np.float32)
            maskcats[c][:, j * NTILE:(j + 1) * NTILE] = row[None, :]

    use_bias = bool(np.any(b1) or np.any(b2) or np.any(b3) or np.any(b4))

    key = (sched, use_bias, n_core)
    if key not in _BUILD_CACHE:
        _BUILD_CACHE[key] = _build(sched, use_bias, n_core)
    nc = _BUILD_CACHE[key]

    # ---- weight tensors in lhsT layouts
    w1h = np.zeros((8, 512), np.float32)
    w1h[0:6, 0:256] = W1                      # full: state,qop,dz_sub
    w1h[0:5, 256:512] = W1[0:5]               # partial: dz slot zeroed,
    w1h[6, 256:512] = W1[5]                   # dz weight reads dz_partial row
    w2h = np.concatenate([W2[0:128], W2[128:256]], axis=1).astype(np.float32)
    w3h = np.concatenate([W3[0:128], W3[128:256]], axis=1).astype(np.float32)
    w4h = np.concatenate([W4[0:128], W4[128:256]], axis=1).astype(np.float32)

    in_map = {"w1": w1h, "w2": w2h, "w3": w3h, "w4": w4h}
    if use_bias:
        b123 = np.stack([b1[0:128], b1[128:256], b2[0:128], b2[128:256],
                         b3[0:128], b3[128:256]], axis=1).astype(np.float32)
        in_map["b123"] = b123
        in_map["b4r"] = b4.reshape(4, 1).astype(np.float32)

    in_maps = [{**in_map, "xm": xms[c], "maskcat": maskcats[c]}
               for c in range(NCORES)]

    trace = os.environ.get("BASSK_TRACE") == "1" and _install_ntff_hook()
    try:
        res = run_bass_kernel_spmd(nc, in_maps, list(range(NCORES)), trace=trace)
    except Exception:
        if not trace:
            raise
        res = run_bass_kernel_spmd(nc, in_maps, list(range(NCORES)), trace=False)
    LAST_EXEC_NS = res.exec_time_ns

    out = np.empty((B, 4), np.float32)
    for c in range(NCORES):
        out[perms[c], :] = res.results[c]["outT"].T
    return out


# revision 48
# speedup vs baseline: 1.0012x; 1.0012x over previous
"""Trainium2 Bass kernel for CompositionalPINN forward.

Reference semantics (B=262144, H=256, N_STEPS=8):
    state = state_dz[:, :4]; qop = state_dz[:, 4:5]; dz_sub = state_dz[:, 5:6]/8
    n_full = floor(z_frac*8); frac = z_frac*8 - n_full
    for step in range(8):
        state += (n_full > step) * MLP(state, qop, dz_sub)        # residual MLP
    state += (frac > 1e-6) * MLP(state, qop, frac*dz_sub)
    MLP(x) = silu(silu(silu(x@W1+b1)@W2+b2)@W3+b3)@W4+b4  (6->256->256->256->4)

Strategy: pure data parallel over 8 cores.  Host transposes inputs to a
feature-major layout, precomputes the per-sample step masks, and sorts
samples by n_full (descending, dealt round-robin across cores) so each
512-sample tile only runs max(n_full)+1 MLP evals instead of 9.  The
per-tile eval schedule is baked into the compiled program (derived from
the input data; recompiles if the schedule signature changes, with an
in-process cache).

Device pipelining: each 4-tile group runs as two half-groups A/B half
a round out of phase, with per-layer stage interleaving.  B's matmuls
fill the PE queue while A's silus run on the Scalar engine and vice
versa, and each half-group's L4 + state update is deferred into the
opposite phase (and across group boundaries, draining once at program
end), so the in-order PE rarely head-blocks on a just-issued silu.
This matters doubly on TRN2: the PE only reaches its full 2.4 GHz
clock after ~3us of uninterrupted work (1.2 GHz otherwise), so every
stall also downclocks the subsequent matmul streak.  PSUM is fully
used as 4 single-buffer [128,1024] pools (8 banks): h1/h2 rotate
write-over-read in one pool pair (the rotation deps coincide with the
true pipeline deps), h3 lives in the other pair until the deferred L4
accumulates the [4,512] delta into its subregion after silu3 consumed
it — no extra banks, no false dependencies.  2161us -> 1129us vs the
eval-contiguous baseline.

Per eval (tile of 512 samples, feature-major):
  - DVE copy xm[0:8, tile] -> x16r (float32r rounding for the PE; the
    BIR verifier requires f32r matmul operands from a rounding op)
  - L1: 2 matmuls  K=8 (state,qop,dz[,dz_partial]) -> psum [128,1024]
  - silu on ACT over both psum banks in one op -> sbuf float32r
  - L2/L3: 4 matmuls each (K=128 x2 accumulate, M=128 x2)
  - L4: 2 matmuls -> delta in h3-psum[0:4, 0:512]
  - masked evals: DVE mul with a host-precomputed 4-row replicated
    mask slot; plain evals skip all mask work
  - DVE add of delta into the fp32 state rows

float32r runs the PE at full rate (1 cycle/row vs 4 for fp32; N>=256
required); state accumulation stays fp32 in SBUF.  SILU on the Scalar
engine is ~1us per [128,1024] (1 col/cycle at 1.2 GHz, dtype
independent) — 3 silus x ~288 evals ~= 870us is the scalar-engine
floor for this schedule and the binding roofline together with the
PE's ~800us of full-clock matmul work.
"""

import numpy as np
from contextlib import ExitStack

import concourse.bass as bass
import concourse.tile as tile
from concourse import bacc, mybir
from concourse.bass_utils import run_bass_kernel_spmd

F32 = mybir.dt.float32
F32R = mybir.dt.float32r
Silu = mybir.ActivationFunctionType.Silu

NCORES = 8
NTILE = 512
CHUNK_TILES = 4                     # tiles per DMA chunk
H = 256
NSTEPS = 8

# xm row layout (row 7 is a zero spare; its L1 weight rows are zero)
R_STATE = 0          # rows 0-3
R_QOP = 4
R_DZSUB = 5
R_DZPART = 6


GROUP = 4                           # tiles pipelined per round


def _round_iter(schedule, tiles):
    """Yield rounds: lists of (tile_index, eval_desc) live this round.
    Used by both the program builder and the host maskcat packer — the
    (round, tile) iteration order must stay identical."""
    chunks = [(c0, min(c0 + CHUNK_TILES, tiles))
              for c0 in range(0, tiles, CHUNK_TILES)]
    for (c0, c1) in chunks:
        for g0 in range(c0, c1, GROUP):
            group = list(range(g0, min(g0 + GROUP, c1)))
            maxev = max((len(schedule[t]) for t in group), default=0)
            for r in range(maxev):
                yield [(t, schedule[t][r]) for t in group
                       if r < len(schedule[t])]


def _emit_order(schedule, tiles):
    for rnd in _round_iter(schedule, tiles):
        yield from rnd


_BUILD_CACHE = {}

LAST_EXEC_NS = None  # set when BASSK_TRACE=1


def _install_ntff_hook():
    """The agent image lacks antenv.axon_hooks; synthesize it so
    run_bass_kernel_spmd(trace=True) can reach the NTFF profiler."""
    import sys
    import types
    if "antenv.axon_hooks" in sys.modules:
        return True
    try:
        import antenv
        from trn_agent_boot.trn_boot import _ntff_profile_via_ctypes
        hook = _ntff_profile_via_ctypes("/opt/axon/libaxon_pjrt.so")
        if hook is None:
            return False
        mod = types.ModuleType("antenv.axon_hooks")
        mod.get_axon_ntff_profile_hook = lambda: hook
        mod.set_axon_ntff_profile_hook = lambda h: None
        sys.modules["antenv.axon_hooks"] = mod
        antenv.axon_hooks = mod
        return True
    except Exception:
        return False


def _build(schedule, use_bias, n_core):
    """schedule: tuple over tiles of tuples of (is_partial, use_mask).

    Masked evals read consecutive NTILE-wide slots of the packed
    per-core mask tensor, in schedule order."""
    tiles = n_core // NTILE
    n_masked = sum(1 for tev in schedule for (_, m) in tev if m)
    nc = bacc.Bacc("TRN2", target_bir_lowering=False, debug=False,
                   num_devices=NCORES)

    xm_d = nc.declare_dram_parameter("xm", [8, n_core], F32, isOutput=False)
    mk_d = nc.declare_dram_parameter("maskcat", [4, max(1, n_masked) * NTILE],
                                     F32, isOutput=False)
    w1_d = nc.declare_dram_parameter("w1", [8, 512], F32, isOutput=False)
    w2_d = nc.declare_dram_parameter("w2", [128, 512], F32, isOutput=False)
    w3_d = nc.declare_dram_parameter("w3", [128, 512], F32, isOutput=False)
    w4_d = nc.declare_dram_parameter("w4", [128, 8], F32, isOutput=False)
    if use_bias:
        b123_d = nc.declare_dram_parameter("b123", [128, 6], F32, isOutput=False)
        b4_d = nc.declare_dram_parameter("b4r", [4, 1], F32, isOutput=False)
    out_d = nc.declare_dram_parameter("outT", [4, n_core], F32, isOutput=True)

    chunks = [(c0, min(c0 + CHUNK_TILES, tiles))
              for c0 in range(0, tiles, CHUNK_TILES)]

    with tile.TileContext(nc) as tc, ExitStack() as ctx:
        const = ctx.enter_context(tc.tile_pool(name="const", bufs=1))
        data = ctx.enter_context(tc.tile_pool(name="data", bufs=1))
        acts = ctx.enter_context(tc.tile_pool(name="acts", bufs=10))
        xr = ctx.enter_context(tc.tile_pool(name="xr", bufs=4))
        tmp = ctx.enter_context(tc.tile_pool(name="tmp", bufs=2))
        # One single-buffer psum pool per (stage position, role), created
        # interleaved so a stage-pair's two h tiles sit 4 banks apart —
        # h1/h2 rotate in pA/pB (write-over-read is the true pipeline
        # dep), h3 tiles live longer (until the deferred L4+add of the
        # opposite phase) in qA/qB.
        pA = ctx.enter_context(tc.tile_pool(name="pA", bufs=1, space="PSUM"))
        qA = ctx.enter_context(tc.tile_pool(name="qA", bufs=1, space="PSUM"))
        pB = ctx.enter_context(tc.tile_pool(name="pB", bufs=1, space="PSUM"))
        qB = ctx.enter_context(tc.tile_pool(name="qB", bufs=1, space="PSUM"))
        ps12 = [pA, pB]
        ps3 = [qA, qB]

        # ---- weights: DMA fp32 staging, DVE round to float32r
        w1_s = const.tile([8, 512], F32)
        nc.gpsimd.dma_start(out=w1_s, in_=w1_d[:, :])
        w1 = const.tile([8, 512], F32R)
        nc.gpsimd.tensor_copy(w1, w1_s)
        w2_s = const.tile([128, 512], F32)
        nc.gpsimd.dma_start(out=w2_s, in_=w2_d[:, :])
        w2 = const.tile([128, 512], F32R)
        nc.gpsimd.tensor_copy(w2, w2_s)
        w3_s = const.tile([128, 512], F32)
        nc.gpsimd.dma_start(out=w3_s, in_=w3_d[:, :])
        w3 = const.tile([128, 512], F32R)
        nc.gpsimd.tensor_copy(w3, w3_s)
        w4_s = const.tile([128, 8], F32)
        nc.gpsimd.dma_start(out=w4_s, in_=w4_d[:, :])
        w4 = const.tile([128, 8], F32R)
        nc.gpsimd.tensor_copy(w4, w4_s)
        if use_bias:
            b123 = const.tile([128, 6], F32)
            nc.gpsimd.dma_start(out=b123, in_=b123_d[:, :])
            b4r = const.tile([4, 1], F32)
            nc.gpsimd.dma_start(out=b4r, in_=b4_d[:, :])

        # ---- the full per-core dataset stays resident in SBUF
        xm = data.tile([8, n_core], F32)
        mkc = data.tile([4, max(1, n_masked) * NTILE], F32)
        nc.gpsimd.dma_start(out=mkc, in_=mk_d[:, :])
        mask_slot = [0]
        for (c0, c1) in chunks:
            nc.sync.dma_start(out=xm[:, c0 * NTILE:c1 * NTILE],
                              in_=xm_d[:, c0 * NTILE:c1 * NTILE])

        # ---- emission helpers over lists of (t, ts, isp, um).
        xs, h1ps, h1ss, h2ps, h2ss, h3ps, h3ss = {}, {}, {}, {}, {}, {}, {}

        def e_cast(live):
            # f32r input snapshot (the BIR verifier requires f32r matmul
            # operands to come from a rounding producer)
            for (t, ts, isp, um) in live:
                x16r = xr.tile([8, NTILE], F32R, tag="x16")
                nc.vector.tensor_copy(x16r, xm[:, ts])
                xs[t] = x16r

        def e_l1(live):
            for i, (t, ts, isp, um) in enumerate(live):
                w1off = 256 if isp else 0
                h1p = ps12[i % 2].tile([128, 2 * NTILE], F32, tag="h")
                nc.tensor.matmul(h1p[:, 0:NTILE], w1[:, w1off:w1off + 128],
                                 xs[t][0:8, :], start=True, stop=True)
                nc.tensor.matmul(h1p[:, NTILE:2 * NTILE],
                                 w1[:, w1off + 128:w1off + 256],
                                 xs[t][0:8, :], start=True, stop=True)
                if use_bias:
                    nc.vector.tensor_scalar_add(h1p[:, 0:NTILE], h1p[:, 0:NTILE], b123[:, 0:1])
                    nc.vector.tensor_scalar_add(h1p[:, NTILE:], h1p[:, NTILE:], b123[:, 1:2])
                h1ps[t] = h1p

        def e_silu(live, src, dst):
            for (t, ts, isp, um) in live:
                hs = acts.tile([128, 2 * NTILE], F32R, tag="h")
                nc.scalar.activation(hs, src[t], Silu)
                dst[t] = hs

        def e_l23(live, w, src, dst, pool, boff):
            for i, (t, ts, isp, um) in enumerate(live):
                hp = pool[i % 2].tile([128, 2 * NTILE], F32, tag="h")
                for mt in range(2):
                    for kt in range(2):
                        nc.tensor.matmul(
                            hp[:, mt * NTILE:(mt + 1) * NTILE],
                            w[:, kt * 256 + mt * 128: kt * 256 + (mt + 1) * 128],
                            src[t][:, kt * NTILE:(kt + 1) * NTILE],
                            start=(kt == 0), stop=(kt == 1))
                if use_bias:
                    nc.vector.tensor_scalar_add(hp[:, 0:NTILE], hp[:, 0:NTILE], b123[:, boff:boff + 1])
                    nc.vector.tensor_scalar_add(hp[:, NTILE:], hp[:, NTILE:], b123[:, boff + 1:boff + 2])
                dst[t] = hp

        def e_l4_add(live):
            # L4 into a subregion of the (consumed) h3 psum tile, then the
            # DVE state update.  Masked evals read a host-precomputed 4-row
            # replicated mask slot — plain DVE mul, no broadcast matmul.
            for (t, ts, isp, um) in live:
                d = h3ps[t][0:4, 0:NTILE]
                nc.tensor.matmul(d, w4[:, 0:4], h3ss[t][:, 0:NTILE],
                                 start=True, stop=False)
                nc.tensor.matmul(d, w4[:, 4:8], h3ss[t][:, NTILE:2 * NTILE],
                                 start=False, stop=True)
            for (t, ts, isp, um) in live:
                d = h3ps[t][0:4, 0:NTILE]
                if use_bias:
                    nc.vector.tensor_scalar_add(d, d, b4r[:, 0:1])
                if not um:
                    nc.vector.tensor_add(xm[0:4, ts], xm[0:4, ts], d)
                else:
                    j = mask_slot[0]
                    mask_slot[0] += 1
                    dm = tmp.tile([4, NTILE], F32, tag="dm")
                    nc.vector.tensor_mul(dm, d, mkc[:, j * NTILE:(j + 1) * NTILE])
                    nc.vector.tensor_add(xm[0:4, ts], xm[0:4, ts], dm)

        # ---- two half-groups A/B per 4-tile group run half a round out of
        # phase: B's matmuls keep the PE busy while A's tail silus run, and
        # each half-group's L4+state-update is deferred into the opposite
        # phase so the PE never head-blocks on a just-issued silu.  The
        # flat masked-eval order stays A(0),B(0),A(1),B(1),... — identical
        # to _emit_order.
        def mklive(ts_, r):
            return [(t, bass.ds(t * NTILE, NTILE), *schedule[t][r])
                    for t in ts_ if r < len(schedule[t])]

        # prevB carries ACROSS groups and chunks: the final half-group's
        # deferred L4+add lands inside the next group's first window, so
        # group boundaries never head-block the PE on a tail silu.
        # (Output DMA stays at program end: emitting it mid-stream was
        # measured to serialize the pipeline, +220us.)
        prevB = []
        for (c0, c1) in chunks:
            for g0 in range(c0, c1, GROUP):
                gt = list(range(g0, min(g0 + GROUP, c1)))
                half = (len(gt) + 1) // 2
                A, B = gt[:half], gt[half:]
                R = max((len(schedule[t]) for t in gt), default=0)

                for r in range(R):
                    liveA = mklive(A, r)
                    liveB = mklive(B, r)
                    e_cast(liveA)                              # w1
                    e_l1(liveA)                                # w2
                    e_silu(liveA, h1ps, h1ss)                  # w3
                    e_l23(liveA, w2, h1ss, h2ps, ps12, 2)      # w4
                    e_l4_add(prevB)                            # w5
                    e_cast(liveB)
                    e_silu(liveA, h2ps, h2ss)                  # w6
                    e_l23(liveA, w3, h2ss, h3ps, ps3, 4)       # w7
                    e_silu(liveA, h3ps, h3ss)                  # w8
                    e_l1(liveB)                                # w9
                    e_silu(liveB, h1ps, h1ss)                  # w10
                    e_l23(liveB, w2, h1ss, h2ps, ps12, 2)      # w11
                    e_l4_add(liveA)                            # w12
                    e_silu(liveB, h2ps, h2ss)                  # w13
                    e_l23(liveB, w3, h2ss, h3ps, ps3, 4)       # w14
                    e_silu(liveB, h3ps, h3ss)                  # w15
                    prevB = liveB
        e_l4_add(prevB)                                        # program tail
        for (c0, c1) in chunks:
            nc.sync.dma_start(out=out_d[:, c0 * NTILE:c1 * NTILE],
                              in_=xm[0:4, c0 * NTILE:c1 * NTILE])

    nc.compile()
    return nc


def kernel(state_dz, z_frac, W1, b1, W2, b2, W3, b3, W4, b4):
    global LAST_EXEC_NS
    import os

    state_dz = np.ascontiguousarray(state_dz, dtype=np.float32)
    z_frac = np.ascontiguousarray(z_frac, dtype=np.float32)
    W1 = np.asarray(W1, np.float32); W2 = np.asarray(W2, np.float32)
    W3 = np.asarray(W3, np.float32); W4 = np.asarray(W4, np.float32)
    b1 = np.asarray(b1, np.float32); b2 = np.asarray(b2, np.float32)
    b3 = np.asarray(b3, np.float32); b4 = np.asarray(b4, np.float32)

    B = state_dz.shape[0]
    assert B % (NCORES * NTILE) == 0, f"B={B} must be divisible by {NCORES * NTILE}"
    n_core = B // NCORES
    tiles = n_core // NTILE

    # ---- host-side derived quantities (bitwise-identical fp32 ops vs jax)
    dz_sub = (state_dz[:, 5] / np.float32(8.0)).astype(np.float32)
    cont = (z_frac * np.float32(NSTEPS)).astype(np.float32)
    n_full = np.floor(cont).astype(np.float32)
    frac = (cont - n_full).astype(np.float32)
    dz_part = (frac * dz_sub).astype(np.float32)
    has_part = (frac > np.float32(1e-6)).astype(np.float32)
    n_int = np.minimum(n_full, NSTEPS).astype(np.int64)

    # ---- sort desc by n_full, deal round-robin to cores
    order = np.argsort(-n_int, kind="stable")
    perms = [order[c::NCORES] for c in range(NCORES)]

    # ---- build per-core xm arrays  [16, N_CORE]
    xms = []
    for c in range(NCORES):
        p = perms[c]
        xm = np.zeros((8, n_core), np.float32)
        xm[0:4] = state_dz[p, 0:4].T
        xm[R_QOP] = state_dz[p, 4]
        xm[R_DZSUB] = dz_sub[p]
        xm[R_DZPART] = dz_part[p]
        xms.append(xm)

    # ---- union schedule across cores (SPMD: one program for all cores)
    sched = []
    for t in range(tiles):
        sl = slice(t * NTILE, (t + 1) * NTILE)
        smax, smin = 0, NSTEPS
        anyp, allp = False, True
        for c in range(NCORES):
            nf = n_int[perms[c][sl]]
            smax = max(smax, int(nf.max()))
            smin = min(smin, int(nf.min()))
            hp = has_part[perms[c][sl]]
            anyp = anyp or bool(hp.any())
            allp = allp and bool(hp.all())
        evals = []
        steps = 0
        for s in range(min(smax, NSTEPS)):
            evals.append((False, smin <= s))
            steps += 1
        if anyp:
            evals.append((True, not allp))
        sched.append(tuple(evals))
    sched = tuple(sched)

    # masked evals in DEVICE EMISSION order; track each tile's step counter
    masked_evals = []
    step_no = [0] * tiles
    for t, (is_partial, use_mask) in _emit_order(sched, tiles):
        s = None if is_partial else step_no[t]
        if not is_partial:
            step_no[t] += 1
        if use_mask:
            masked_evals.append((t, s))

    # ---- packed mask rows (replicated to 4 partitions), one NTILE slot
    # per masked eval, per core
    nm = max(1, len(masked_evals))
    maskcats = [np.zeros((4, nm * NTILE), np.float32) for _ in range(NCORES)]
    for j, (t, s) in enumerate(masked_evals):
        sl = slice(t * NTILE, (t + 1) * NTILE)
        for c in range(NCORES):
            idx = perms[c][sl]
            row = has_part[idx] if s is None else (n_full[idx] > s).astype(np.float32)
            maskcats[c][:, j * NTILE:(j + 1) * NTILE] = row[None, :]

    use_bias = bool(np.any(b1) or np.any(b2) or np.any(b3) or np.any(b4))

    key = (sched, use_bias, n_core)
    if key not in _BUILD_CACHE:
        _BUILD_CACHE[key] = _build(sched, use_bias, n_core)
    nc = _BUILD_CACHE[key]

    # ---- weight tensors in lhsT layouts
    w1h = np.zeros((8, 512), np.float32)
    w1h[0:6, 0:256] = W1                      # full: state,qop,dz_sub
    w1h[0:5, 256:512] = W1[0:5]               # partial: dz slot zeroed,
    w1h[6, 256:512] = W1[5]                   # dz weight reads dz_partial row
    w2h = np.concatenate([W2[0:128], W2[128:256]], axis=1).astype(np.float32)
    w3h = np.concatenate([W3[0:128], W3[128:256]], axis=1).astype(np.float32)
    w4h = np.concatenate([W4[0:128], W4[128:256]], axis=1).astype(np.float32)

    in_map = {"w1": w1h, "w2": w2h, "w3": w3h, "w4": w4h}
    if use_bias:
        b123 = np.stack([b1[0:128], b1[128:256], b2[0:128], b2[128:256],
                         b3[0:128], b3[128:256]], axis=1).astype(np.float32)
        in_map["b123"] = b123
        in_map["b4r"] = b4.reshape(4, 1).astype(np.float32)

    in_maps = [{**in_map, "xm": xms[c], "maskcat": maskcats[c]}
               for c in range(NCORES)]

    trace = os.environ.get("BASSK_TRACE") == "1" and _install_ntff_hook()
    try:
        res = run_bass_kernel_spmd(nc, in_maps, list(range(NCORES)), trace=trace)
    except Exception:
        if not trace:
            raise
        res = run_bass_kernel_spmd(nc, in_maps, list(range(NCORES)), trace=False)
    LAST_EXEC_NS = res.exec_time_ns

    out = np.empty((B, 4), np.float32)
    for c in range(NCORES):
        out[perms[c], :] = res.results[c]["outT"].T
    return out


# revision 49
# speedup vs baseline: 1.0885x; 1.0872x over previous
"""Trainium2 Bass kernel for CompositionalPINN forward.

Reference semantics (B=262144, H=256, N_STEPS=8):
    state = state_dz[:, :4]; qop = state_dz[:, 4:5]; dz_sub = state_dz[:, 5:6]/8
    n_full = floor(z_frac*8); frac = z_frac*8 - n_full
    for step in range(8):
        state += (n_full > step) * MLP(state, qop, dz_sub)        # residual MLP
    state += (frac > 1e-6) * MLP(state, qop, frac*dz_sub)
    MLP(x) = silu(silu(silu(x@W1+b1)@W2+b2)@W3+b3)@W4+b4  (6->256->256->256->4)

Strategy: pure data parallel over 8 cores.  Host transposes inputs to a
feature-major layout, precomputes the per-sample step masks, and sorts
samples by n_full (descending, dealt round-robin across cores) so each
512-sample tile only runs max(n_full)+1 MLP evals instead of 9.  The
per-tile eval schedule is baked into the compiled program (derived from
the input data; recompiles if the schedule signature changes, with an
in-process cache).

Device pipelining: each 4-tile group runs as two half-groups A/B half
a round out of phase, with per-layer stage interleaving.  B's matmuls
fill the PE queue while A's silus run on the Scalar engine and vice
versa, and each half-group's L4 + state update is deferred into the
opposite phase (and across group boundaries, draining once at program
end), so the in-order PE rarely head-blocks on a just-issued silu.
This matters doubly on TRN2: the PE only reaches its full 2.4 GHz
clock after ~3us of uninterrupted work (1.2 GHz otherwise), so every
stall also downclocks the subsequent matmul streak.  PSUM is fully
used as 4 single-buffer [128,1024] pools (8 banks): h1/h2 rotate
write-over-read in one pool pair (the rotation deps coincide with the
true pipeline deps), h3 lives in the other pair until the deferred L4
accumulates the [4,512] delta into its subregion after silu3 consumed
it — no extra banks, no false dependencies.  2161us -> 1129us vs the
eval-contiguous baseline.

Per eval (tile of 512 samples, feature-major):
  - DVE copy xm[0:8, tile] -> x16r (float32r rounding for the PE; the
    BIR verifier requires f32r matmul operands from a rounding op)
  - L1: 2 matmuls  K=8 (state,qop,dz[,dz_partial]) -> psum [128,1024]
  - silu on ACT over both psum banks in one op -> sbuf float32r
  - L2/L3: 4 matmuls each (K=128 x2 accumulate, M=128 x2)
  - L4: 2 matmuls -> delta in h3-psum[0:4, 0:512]
  - masked evals: DVE mul with a host-precomputed 4-row replicated
    mask slot; plain evals skip all mask work
  - DVE add of delta into the fp32 state rows

float32r runs the PE at full rate (1 cycle/row vs 4 for fp32; N>=256
required); state accumulation stays fp32 in SBUF.  SILU on the Scalar
engine is ~1us per [128,1024] (1 col/cycle at 1.2 GHz, dtype
independent) — 3 silus x ~288 evals ~= 870us is the scalar-engine
floor for this schedule and the binding roofline together with the
PE's ~800us of full-clock matmul work.
"""

import numpy as np
from contextlib import ExitStack

import concourse.bass as bass
import concourse.tile as tile
from concourse import bacc, mybir
from concourse.bass_utils import run_bass_kernel_spmd

F32 = mybir.dt.float32
F32R = mybir.dt.float32r
BF16 = mybir.dt.bfloat16
Silu = mybir.ActivationFunctionType.Silu

NCORES = 8
NTILE = 512
CHUNK_TILES = 4                     # tiles per DMA chunk
H = 256
NSTEPS = 8

# xm row layout (row 7 is a zero spare; its L1 weight rows are zero)
R_STATE = 0          # rows 0-3
R_QOP = 4
R_DZSUB = 5
R_DZPART = 6


GROUP = 4                           # tiles pipelined per round


def _round_iter(schedule, tiles):
    """Yield rounds: lists of (tile_index, eval_desc) live this round.
    Used by both the program builder and the host maskcat packer — the
    (round, tile) iteration order must stay identical."""
    chunks = [(c0, min(c0 + CHUNK_TILES, tiles))
              for c0 in range(0, tiles, CHUNK_TILES)]
    for (c0, c1) in chunks:
        for g0 in range(c0, c1, GROUP):
            group = list(range(g0, min(g0 + GROUP, c1)))
            maxev = max((len(schedule[t]) for t in group), default=0)
            for r in range(maxev):
                yield [(t, schedule[t][r]) for t in group
                       if r < len(schedule[t])]


def _emit_order(schedule, tiles):
    for rnd in _round_iter(schedule, tiles):
        yield from rnd


_BUILD_CACHE = {}

LAST_EXEC_NS = None  # set when BASSK_TRACE=1


def _install_ntff_hook():
    """The agent image lacks antenv.axon_hooks; synthesize it so
    run_bass_kernel_spmd(trace=True) can reach the NTFF profiler."""
    import sys
    import types
    if "antenv.axon_hooks" in sys.modules:
        return True
    try:
        import antenv
        from trn_agent_boot.trn_boot import _ntff_profile_via_ctypes
        hook = _ntff_profile_via_ctypes("/opt/axon/libaxon_pjrt.so")
        if hook is None:
            return False
        mod = types.ModuleType("antenv.axon_hooks")
        mod.get_axon_ntff_profile_hook = lambda: hook
        mod.set_axon_ntff_profile_hook = lambda h: None
        sys.modules["antenv.axon_hooks"] = mod
        antenv.axon_hooks = mod
        return True
    except Exception:
        return False


def _build(schedule, use_bias, n_core):
    """schedule: tuple over tiles of tuples of (is_partial, use_mask).

    Masked evals read consecutive NTILE-wide slots of the packed
    per-core mask tensor, in schedule order."""
    tiles = n_core // NTILE
    n_masked = sum(1 for tev in schedule for (_, m) in tev if m)
    nc = bacc.Bacc("TRN2", target_bir_lowering=False, debug=False,
                   num_devices=NCORES)

    xm_d = nc.declare_dram_parameter("xm", [8, n_core], F32, isOutput=False)
    mk_d = nc.declare_dram_parameter("maskcat", [4, max(1, n_masked) * NTILE],
                                     F32, isOutput=False)
    w1_d = nc.declare_dram_parameter("w1", [8, 512], F32, isOutput=False)
    w2_d = nc.declare_dram_parameter("w2", [128, 512], F32, isOutput=False)
    w3_d = nc.declare_dram_parameter("w3", [128, 512], F32, isOutput=False)
    w4_d = nc.declare_dram_parameter("w4", [128, 8], F32, isOutput=False)
    if use_bias:
        b123_d = nc.declare_dram_parameter("b123", [128, 6], F32, isOutput=False)
        b4_d = nc.declare_dram_parameter("b4r", [4, 1], F32, isOutput=False)
    out_d = nc.declare_dram_parameter("outT", [4, n_core], F32, isOutput=True)

    chunks = [(c0, min(c0 + CHUNK_TILES, tiles))
              for c0 in range(0, tiles, CHUNK_TILES)]

    with tile.TileContext(nc) as tc, ExitStack() as ctx:
        const = ctx.enter_context(tc.tile_pool(name="const", bufs=1))
        data = ctx.enter_context(tc.tile_pool(name="data", bufs=1))
        acts = ctx.enter_context(tc.tile_pool(name="acts", bufs=10))
        xr = ctx.enter_context(tc.tile_pool(name="xr", bufs=4))
        tmp = ctx.enter_context(tc.tile_pool(name="tmp", bufs=2))
        # One single-buffer psum pool per (stage position, role), created
        # interleaved so a stage-pair's two h tiles sit 4 banks apart —
        # h1/h2 rotate in pA/pB (write-over-read is the true pipeline
        # dep), h3 tiles live longer (until the deferred L4+add of the
        # opposite phase) in qA/qB.
        pA = ctx.enter_context(tc.tile_pool(name="pA", bufs=1, space="PSUM"))
        qA = ctx.enter_context(tc.tile_pool(name="qA", bufs=1, space="PSUM"))
        pB = ctx.enter_context(tc.tile_pool(name="pB", bufs=1, space="PSUM"))
        qB = ctx.enter_context(tc.tile_pool(name="qB", bufs=1, space="PSUM"))
        ps12 = [pA, pB]
        ps3 = [qA, qB]

        # ---- weights: DMA fp32 staging, DVE round to float32r
        w1_s = const.tile([8, 512], F32)
        nc.gpsimd.dma_start(out=w1_s, in_=w1_d[:, :])
        w1 = const.tile([8, 512], BF16)
        nc.gpsimd.tensor_copy(w1, w1_s)
        w2_s = const.tile([128, 512], F32)
        nc.gpsimd.dma_start(out=w2_s, in_=w2_d[:, :])
        w2 = const.tile([128, 512], BF16)
        nc.gpsimd.tensor_copy(w2, w2_s)
        w3_s = const.tile([128, 512], F32)
        nc.gpsimd.dma_start(out=w3_s, in_=w3_d[:, :])
        w3 = const.tile([128, 512], BF16)
        nc.gpsimd.tensor_copy(w3, w3_s)
        w4_s = const.tile([128, 8], F32)
        nc.gpsimd.dma_start(out=w4_s, in_=w4_d[:, :])
        w4 = const.tile([128, 8], BF16)
        nc.gpsimd.tensor_copy(w4, w4_s)
        if use_bias:
            b123 = const.tile([128, 6], F32)
            nc.gpsimd.dma_start(out=b123, in_=b123_d[:, :])
            b4r = const.tile([4, 1], F32)
            nc.gpsimd.dma_start(out=b4r, in_=b4_d[:, :])

        # ---- the full per-core dataset stays resident in SBUF
        xm = data.tile([8, n_core], F32)
        mkc = data.tile([4, max(1, n_masked) * NTILE], F32)
        nc.gpsimd.dma_start(out=mkc, in_=mk_d[:, :])
        mask_slot = [0]
        for (c0, c1) in chunks:
            nc.sync.dma_start(out=xm[:, c0 * NTILE:c1 * NTILE],
                              in_=xm_d[:, c0 * NTILE:c1 * NTILE])

        # ---- emission helpers over lists of (t, ts, isp, um).
        xs, h1ps, h1ss, h2ps, h2ss, h3ps, h3ss = {}, {}, {}, {}, {}, {}, {}

        def e_cast(live):
            # f32r input snapshot (the BIR verifier requires f32r matmul
            # operands to come from a rounding producer)
            for (t, ts, isp, um) in live:
                x16r = xr.tile([8, NTILE], BF16, tag="x16")
                nc.vector.tensor_copy(x16r, xm[:, ts])
                xs[t] = x16r

        def e_l1(live):
            for i, (t, ts, isp, um) in enumerate(live):
                w1off = 256 if isp else 0
                h1p = ps12[i % 2].tile([128, 2 * NTILE], F32, tag="h")
                nc.tensor.matmul(h1p[:, 0:NTILE], w1[:, w1off:w1off + 128],
                                 xs[t][0:8, :], start=True, stop=True)
                nc.tensor.matmul(h1p[:, NTILE:2 * NTILE],
                                 w1[:, w1off + 128:w1off + 256],
                                 xs[t][0:8, :], start=True, stop=True)
                if use_bias:
                    nc.vector.tensor_scalar_add(h1p[:, 0:NTILE], h1p[:, 0:NTILE], b123[:, 0:1])
                    nc.vector.tensor_scalar_add(h1p[:, NTILE:], h1p[:, NTILE:], b123[:, 1:2])
                h1ps[t] = h1p

        def e_silu(live, src, dst):
            for (t, ts, isp, um) in live:
                hs = acts.tile([128, 2 * NTILE], BF16, tag="h")
                nc.scalar.activation(hs, src[t], Silu)
                dst[t] = hs

        def e_l23(live, w, src, dst, pool, boff):
            for i, (t, ts, isp, um) in enumerate(live):
                hp = pool[i % 2].tile([128, 2 * NTILE], F32, tag="h")
                for mt in range(2):
                    for kt in range(2):
                        nc.tensor.matmul(
                            hp[:, mt * NTILE:(mt + 1) * NTILE],
                            w[:, kt * 256 + mt * 128: kt * 256 + (mt + 1) * 128],
                            src[t][:, kt * NTILE:(kt + 1) * NTILE],
                            start=(kt == 0), stop=(kt == 1))
                if use_bias:
                    nc.vector.tensor_scalar_add(hp[:, 0:NTILE], hp[:, 0:NTILE], b123[:, boff:boff + 1])
                    nc.vector.tensor_scalar_add(hp[:, NTILE:], hp[:, NTILE:], b123[:, boff + 1:boff + 2])
                dst[t] = hp

        def e_l4_add(live):
            # L4 into a subregion of the (consumed) h3 psum tile, then the
            # DVE state update.  Masked evals read a host-precomputed 4-row
            # replicated mask slot — plain DVE mul, no broadcast matmul.
            for (t, ts, isp, um) in live:
                d = h3ps[t][0:4, 0:NTILE]
                nc.tensor.matmul(d, w4[:, 0:4], h3ss[t][:, 0:NTILE],
                                 start=True, stop=False)
                nc.tensor.matmul(d, w4[:, 4:8], h3ss[t][:, NTILE:2 * NTILE],
                                 start=False, stop=True)
            for (t, ts, isp, um) in live:
                d = h3ps[t][0:4, 0:NTILE]
                if use_bias:
                    nc.vector.tensor_scalar_add(d, d, b4r[:, 0:1])
                if not um:
                    nc.vector.tensor_add(xm[0:4, ts], xm[0:4, ts], d)
                else:
                    j = mask_slot[0]
                    mask_slot[0] += 1
                    dm = tmp.tile([4, NTILE], F32, tag="dm")
                    nc.vector.tensor_mul(dm, d, mkc[:, j * NTILE:(j + 1) * NTILE])
                    nc.vector.tensor_add(xm[0:4, ts], xm[0:4, ts], dm)

        # ---- two half-groups A/B per 4-tile group run half a round out of
        # phase: B's matmuls keep the PE busy while A's tail silus run, and
        # each half-group's L4+state-update is deferred into the opposite
        # phase so the PE never head-blocks on a just-issued silu.  The
        # flat masked-eval order stays A(0),B(0),A(1),B(1),... — identical
        # to _emit_order.
        def mklive(ts_, r):
            return [(t, bass.ds(t * NTILE, NTILE), *schedule[t][r])
                    for t in ts_ if r < len(schedule[t])]

        # prevB carries ACROSS groups and chunks: the final half-group's
        # deferred L4+add lands inside the next group's first window, so
        # group boundaries never head-block the PE on a tail silu.
        # (Output DMA stays at program end: emitting it mid-stream was
        # measured to serialize the pipeline, +220us.)
        prevB = []
        for (c0, c1) in chunks:
            for g0 in range(c0, c1, GROUP):
                gt = list(range(g0, min(g0 + GROUP, c1)))
                half = (len(gt) + 1) // 2
                A, B = gt[:half], gt[half:]
                R = max((len(schedule[t]) for t in gt), default=0)

                for r in range(R):
                    liveA = mklive(A, r)
                    liveB = mklive(B, r)
                    e_cast(liveA)                              # w1
                    e_l1(liveA)                                # w2
                    e_silu(liveA, h1ps, h1ss)                  # w3
                    e_l23(liveA, w2, h1ss, h2ps, ps12, 2)      # w4
                    e_l4_add(prevB)                            # w5
                    e_cast(liveB)
                    e_silu(liveA, h2ps, h2ss)                  # w6
                    e_l23(liveA, w3, h2ss, h3ps, ps3, 4)       # w7
                    e_silu(liveA, h3ps, h3ss)                  # w8
                    e_l1(liveB)                                # w9
                    e_silu(liveB, h1ps, h1ss)                  # w10
                    e_l23(liveB, w2, h1ss, h2ps, ps12, 2)      # w11
                    e_l4_add(liveA)                            # w12
                    e_silu(liveB, h2ps, h2ss)                  # w13
                    e_l23(liveB, w3, h2ss, h3ps, ps3, 4)       # w14
                    e_silu(liveB, h3ps, h3ss)                  # w15
                    prevB = liveB
        e_l4_add(prevB)                                        # program tail
        for (c0, c1) in chunks:
            nc.sync.dma_start(out=out_d[:, c0 * NTILE:c1 * NTILE],
                              in_=xm[0:4, c0 * NTILE:c1 * NTILE])

    nc.compile()
    return nc


def kernel(state_dz, z_frac, W1, b1, W2, b2, W3, b3, W4, b4):
    global LAST_EXEC_NS
    import os

    state_dz = np.ascontiguousarray(state_dz, dtype=np.float32)
    z_frac = np.ascontiguousarray(z_frac, dtype=np.float32)
    W1 = np.asarray(W1, np.float32); W2 = np.asarray(W2, np.float32)
    W3 = np.asarray(W3, np.float32); W4 = np.asarray(W4, np.float32)
    b1 = np.asarray(b1, np.float32); b2 = np.asarray(b2, np.float32)
    b3 = np.asarray(b3, np.float32); b4 = np.asarray(b4, np.float32)

    B = state_dz.shape[0]
    assert B % (NCORES * NTILE) == 0, f"B={B} must be divisible by {NCORES * NTILE}"
    n_core = B // NCORES
    tiles = n_core // NTILE

    # ---- host-side derived quantities (bitwise-identical fp32 ops vs jax)
    dz_sub = (state_dz[:, 5] / np.float32(8.0)).astype(np.float32)
    cont = (z_frac * np.float32(NSTEPS)).astype(np.float32)
    n_full = np.floor(cont).astype(np.float32)
    frac = (cont - n_full).astype(np.float32)
    dz_part = (frac * dz_sub).astype(np.float32)
    has_part = (frac > np.float32(1e-6)).astype(np.float32)
    n_int = np.minimum(n_full, NSTEPS).astype(np.int64)

    # ---- sort desc by n_full, deal round-robin to cores
    order = np.argsort(-n_int, kind="stable")
    perms = [order[c::NCORES] for c in range(NCORES)]

    # ---- build per-core xm arrays  [16, N_CORE]
    xms = []
    for c in range(NCORES):
        p = perms[c]
        xm = np.zeros((8, n_core), np.float32)
        xm[0:4] = state_dz[p, 0:4].T
        xm[R_QOP] = state_dz[p, 4]
        xm[R_DZSUB] = dz_sub[p]
        xm[R_DZPART] = dz_part[p]
        xms.append(xm)

    # ---- union schedule across cores (SPMD: one program for all cores)
    sched = []
    for t in range(tiles):
        sl = slice(t * NTILE, (t + 1) * NTILE)
        smax, smin = 0, NSTEPS
        anyp, allp = False, True
        for c in range(NCORES):
            nf = n_int[perms[c][sl]]
            smax = max(smax, int(nf.max()))
            smin = min(smin, int(nf.min()))
            hp = has_part[perms[c][sl]]
            anyp = anyp or bool(hp.any())
            allp = allp and bool(hp.all())
        evals = []
        steps = 0
        for s in range(min(smax, NSTEPS)):
            evals.append((False, smin <= s))
            steps += 1
        if anyp:
            evals.append((True, not allp))
        sched.append(tuple(evals))
    sched = tuple(sched)

    # masked evals in DEVICE EMISSION order; track each tile's step counter
    masked_evals = []
    step_no = [0] * tiles
    for t, (is_partial, use_mask) in _emit_order(sched, tiles):
        s = None if is_partial else step_no[t]
        if not is_partial:
            step_no[t] += 1
        if use_mask:
            masked_evals.append((t, s))

    # ---- packed mask rows (replicated to 4 partitions), one NTILE slot
    # per masked eval, per core
    nm = max(1, len(masked_evals))
    maskcats = [np.zeros((4, nm * NTILE), np.float32) for _ in range(NCORES)]
    for j, (t, s) in enumerate(masked_evals):
        sl = slice(t * NTILE, (t + 1) * NTILE)
        for c in range(NCORES):
            idx = perms[c][sl]
            row = has_part[idx] if s is None else (n_full[idx] > s).astype(np.float32)
            maskcats[c][:, j * NTILE:(j + 1) * NTILE] = row[None, :]

    use_bias = bool(np.any(b1) or np.any(b2) or np.any(b3) or np.any(b4))

    key = (sched, use_bias, n_core)
    if key not in _BUILD_CACHE:
        _BUILD_CACHE[key] = _build(sched, use_bias, n_core)
    nc = _BUILD_CACHE[key]

    # ---- weight tensors in lhsT layouts
    w1h = np.zeros((8, 512), np.float32)
    w1h[0:6, 0:256] = W1                      # full: state,qop,dz_sub
    w1h[0:5, 256:512] = W1[0:5]               # partial: dz slot zeroed,
    w1h[6, 256:512] = W1[5]                   # dz weight reads dz_partial row
    w2h = np.concatenate([W2[0:128], W2[128:256]], axis=1).astype(np.float32)
    w3h = np.concatenate([W3[0:128], W3[128:256]], axis=1).astype(np.float32)
    w4h = np.concatenate([W4[0:128], W4[128:256]], axis=1).astype(np.float32)

    in_map = {"w1": w1h, "w2": w2h, "w3": w3h, "w4": w4h}
    if use_bias:
        b123 = np.stack([b1[0:128], b1[128:256], b2[0:128], b2[128:256],
                         b3[0:128], b3[128:256]], axis=1).astype(np.float32)
        in_map["b123"] = b123
        in_map["b4r"] = b4.reshape(4, 1).astype(np.float32)

    in_maps = [{**in_map, "xm": xms[c], "maskcat": maskcats[c]}
               for c in range(NCORES)]

    trace = os.environ.get("BASSK_TRACE") == "1" and _install_ntff_hook()
    try:
        res = run_bass_kernel_spmd(nc, in_maps, list(range(NCORES)), trace=trace)
    except Exception:
        if not trace:
            raise
        res = run_bass_kernel_spmd(nc, in_maps, list(range(NCORES)), trace=False)
    LAST_EXEC_NS = res.exec_time_ns

    out = np.empty((B, 4), np.float32)
    for c in range(NCORES):
        out[perms[c], :] = res.results[c]["outT"].T
    return out


# revision 51
# speedup vs baseline: 1.0895x; 1.0009x over previous
"""Trainium2 Bass kernel for CompositionalPINN forward.

Reference semantics (B=262144, H=256, N_STEPS=8):
    state = state_dz[:, :4]; qop = state_dz[:, 4:5]; dz_sub = state_dz[:, 5:6]/8
    n_full = floor(z_frac*8); frac = z_frac*8 - n_full
    for step in range(8):
        state += (n_full > step) * MLP(state, qop, dz_sub)        # residual MLP
    state += (frac > 1e-6) * MLP(state, qop, frac*dz_sub)
    MLP(x) = silu(silu(silu(x@W1+b1)@W2+b2)@W3+b3)@W4+b4  (6->256->256->256->4)

Strategy: pure data parallel over 8 cores.  Host transposes inputs to a
feature-major layout, precomputes the per-sample step masks, and sorts
samples by n_full (descending, dealt round-robin across cores) so each
512-sample tile only runs max(n_full)+1 MLP evals instead of 9.  The
per-tile eval schedule is baked into the compiled program (derived from
the input data; recompiles if the schedule signature changes, with an
in-process cache).

Device pipelining: each 4-tile group runs as two half-groups A/B half
a round out of phase, with per-layer stage interleaving.  B's matmuls
fill the PE queue while A's silus run on the Scalar engine and vice
versa, and each half-group's L4 + state update is deferred into the
opposite phase (and across group boundaries, draining once at program
end), so the in-order PE rarely head-blocks on a just-issued silu.
This matters doubly on TRN2: the PE only reaches its full 2.4 GHz
clock after ~3us of uninterrupted work (1.2 GHz otherwise), so every
stall also downclocks the subsequent matmul streak.  PSUM is fully
used as 4 single-buffer [128,1024] pools (8 banks): h1/h2 rotate
write-over-read in one pool pair (the rotation deps coincide with the
true pipeline deps), h3 lives in the other pair until the deferred L4
accumulates the [4,512] delta into its subregion after silu3 consumed
it — no extra banks, no false dependencies.  2161us -> 1129us vs the
eval-contiguous baseline.

Per eval (tile of 512 samples, feature-major):
  - DVE copy xm[0:8, tile] -> x16r bf16 (matmul moving operand)
  - L1: 2 matmuls  K=8 (state,qop,dz[,dz_partial]) -> psum [128,1024]
  - silu on ACT over both psum banks in one op -> sbuf bf16
  - L2/L3: 4 matmuls each (K=128 x2 accumulate, M=128 x2)
  - L4: 2 matmuls -> delta in h3-psum[0:4, 0:512]
  - masked evals: DVE mul with a host-precomputed 4-row replicated
    mask slot; plain evals skip all mask work
  - DVE add of delta into the fp32 state rows

All matmul operands are bf16 (same 1 cycle/row as float32r, but the
stationary weight load is a 2-byte load instead of f32r's 4-byte one —
halving the serial LDWEIGHTS at every dependency handoff head was
worth ~86us); psum accumulation and the state rows stay fp32, keeping
scale-relative error at ~5e-3 vs the 2e-2 gate.  SILU on the Scalar
engine is ~1us per [128,1024] (1 col/cycle at 1.2 GHz, dtype
independent) — 3 silus x ~288 evals ~= 870us is the scalar-engine
floor for this schedule and now the binding roofline.
"""

import numpy as np
from contextlib import ExitStack

import concourse.bass as bass
import concourse.tile as tile
from concourse import bacc, mybir
from concourse.bass_utils import run_bass_kernel_spmd

F32 = mybir.dt.float32
F32R = mybir.dt.float32r
BF16 = mybir.dt.bfloat16
Silu = mybir.ActivationFunctionType.Silu

NCORES = 8
NTILE = 512
CHUNK_TILES = 4                     # tiles per DMA chunk
H = 256
NSTEPS = 8

# xm row layout (row 7 is a zero spare; its L1 weight rows are zero)
R_STATE = 0          # rows 0-3
R_QOP = 4
R_DZSUB = 5
R_DZPART = 6


GROUP = 4                           # tiles pipelined per round


def _round_iter(schedule, tiles):
    """Yield rounds: lists of (tile_index, eval_desc) live this round.
    Used by both the program builder and the host maskcat packer — the
    (round, tile) iteration order must stay identical."""
    chunks = [(c0, min(c0 + CHUNK_TILES, tiles))
              for c0 in range(0, tiles, CHUNK_TILES)]
    for (c0, c1) in chunks:
        for g0 in range(c0, c1, GROUP):
            group = list(range(g0, min(g0 + GROUP, c1)))
            maxev = max((len(schedule[t]) for t in group), default=0)
            for r in range(maxev):
                yield [(t, schedule[t][r]) for t in group
                       if r < len(schedule[t])]


def _emit_order(schedule, tiles):
    for rnd in _round_iter(schedule, tiles):
        yield from rnd


_BUILD_CACHE = {}

LAST_EXEC_NS = None  # set when BASSK_TRACE=1


def _install_ntff_hook():
    """The agent image lacks antenv.axon_hooks; synthesize it so
    run_bass_kernel_spmd(trace=True) can reach the NTFF profiler."""
    import sys
    import types
    if "antenv.axon_hooks" in sys.modules:
        return True
    try:
        import antenv
        from trn_agent_boot.trn_boot import _ntff_profile_via_ctypes
        hook = _ntff_profile_via_ctypes("/opt/axon/libaxon_pjrt.so")
        if hook is None:
            return False
        mod = types.ModuleType("antenv.axon_hooks")
        mod.get_axon_ntff_profile_hook = lambda: hook
        mod.set_axon_ntff_profile_hook = lambda h: None
        sys.modules["antenv.axon_hooks"] = mod
        antenv.axon_hooks = mod
        return True
    except Exception:
        return False


def _build(schedule, use_bias, n_core):
    """schedule: tuple over tiles of tuples of (is_partial, use_mask).

    Masked evals read consecutive NTILE-wide slots of the packed
    per-core mask tensor, in schedule order."""
    tiles = n_core // NTILE
    n_masked = sum(1 for tev in schedule for (_, m) in tev if m)
    nc = bacc.Bacc("TRN2", target_bir_lowering=False, debug=False,
                   num_devices=NCORES)

    xm_d = nc.declare_dram_parameter("xm", [8, n_core], F32, isOutput=False)
    mk_d = nc.declare_dram_parameter("maskcat", [4, max(1, n_masked) * NTILE],
                                     F32, isOutput=False)
    w1_d = nc.declare_dram_parameter("w1", [8, 512], F32, isOutput=False)
    w2_d = nc.declare_dram_parameter("w2", [128, 512], F32, isOutput=False)
    w3_d = nc.declare_dram_parameter("w3", [128, 512], F32, isOutput=False)
    w4_d = nc.declare_dram_parameter("w4", [128, 8], F32, isOutput=False)
    if use_bias:
        b123_d = nc.declare_dram_parameter("b123", [128, 6], F32, isOutput=False)
        b4_d = nc.declare_dram_parameter("b4r", [4, 1], F32, isOutput=False)
    out_d = nc.declare_dram_parameter("outT", [4, n_core], F32, isOutput=True)

    chunks = [(c0, min(c0 + CHUNK_TILES, tiles))
              for c0 in range(0, tiles, CHUNK_TILES)]

    with tile.TileContext(nc) as tc, ExitStack() as ctx:
        const = ctx.enter_context(tc.tile_pool(name="const", bufs=1))
        data = ctx.enter_context(tc.tile_pool(name="data", bufs=1))
        acts = ctx.enter_context(tc.tile_pool(name="acts", bufs=16))
        xr = ctx.enter_context(tc.tile_pool(name="xr", bufs=6))
        tmp = ctx.enter_context(tc.tile_pool(name="tmp", bufs=2))
        # One single-buffer psum pool per (stage position, role), created
        # interleaved so a stage-pair's two h tiles sit 4 banks apart —
        # h1/h2 rotate in pA/pB (write-over-read is the true pipeline
        # dep), h3 tiles live longer (until the deferred L4+add of the
        # opposite phase) in qA/qB.
        pA = ctx.enter_context(tc.tile_pool(name="pA", bufs=1, space="PSUM"))
        qA = ctx.enter_context(tc.tile_pool(name="qA", bufs=1, space="PSUM"))
        pB = ctx.enter_context(tc.tile_pool(name="pB", bufs=1, space="PSUM"))
        qB = ctx.enter_context(tc.tile_pool(name="qB", bufs=1, space="PSUM"))
        ps12 = [pA, pB]
        ps3 = [qA, qB]

        # ---- weights: DMA fp32 staging, DVE round to float32r
        w1_s = const.tile([8, 512], F32)
        nc.gpsimd.dma_start(out=w1_s, in_=w1_d[:, :])
        w1 = const.tile([8, 512], BF16)
        nc.gpsimd.tensor_copy(w1, w1_s)
        w2_s = const.tile([128, 512], F32)
        nc.gpsimd.dma_start(out=w2_s, in_=w2_d[:, :])
        w2 = const.tile([128, 512], BF16)
        nc.gpsimd.tensor_copy(w2, w2_s)
        w3_s = const.tile([128, 512], F32)
        nc.gpsimd.dma_start(out=w3_s, in_=w3_d[:, :])
        w3 = const.tile([128, 512], BF16)
        nc.gpsimd.tensor_copy(w3, w3_s)
        w4_s = const.tile([128, 8], F32)
        nc.gpsimd.dma_start(out=w4_s, in_=w4_d[:, :])
        w4 = const.tile([128, 8], BF16)
        nc.gpsimd.tensor_copy(w4, w4_s)
        if use_bias:
            b123 = const.tile([128, 6], F32)
            nc.gpsimd.dma_start(out=b123, in_=b123_d[:, :])
            b4r = const.tile([4, 1], F32)
            nc.gpsimd.dma_start(out=b4r, in_=b4_d[:, :])

        # ---- the full per-core dataset stays resident in SBUF
        xm = data.tile([8, n_core], F32)
        mkc = data.tile([4, max(1, n_masked) * NTILE], F32)
        nc.gpsimd.dma_start(out=mkc, in_=mk_d[:, :])
        mask_slot = [0]
        for (c0, c1) in chunks:
            nc.sync.dma_start(out=xm[:, c0 * NTILE:c1 * NTILE],
                              in_=xm_d[:, c0 * NTILE:c1 * NTILE])

        # ---- emission helpers over lists of (t, ts, isp, um).
        xs, h1ps, h1ss, h2ps, h2ss, h3ps, h3ss = {}, {}, {}, {}, {}, {}, {}

        def e_cast(live):
            # f32r input snapshot (the BIR verifier requires f32r matmul
            # operands to come from a rounding producer)
            for (t, ts, isp, um) in live:
                x16r = xr.tile([8, NTILE], BF16, tag="x16")
                nc.vector.tensor_copy(x16r, xm[:, ts])
                xs[t] = x16r

        def e_l1(live):
            for i, (t, ts, isp, um) in enumerate(live):
                w1off = 256 if isp else 0
                h1p = ps12[i % 2].tile([128, 2 * NTILE], F32, tag="h")
                nc.tensor.matmul(h1p[:, 0:NTILE], w1[:, w1off:w1off + 128],
                                 xs[t][0:8, :], start=True, stop=True)
                nc.tensor.matmul(h1p[:, NTILE:2 * NTILE],
                                 w1[:, w1off + 128:w1off + 256],
                                 xs[t][0:8, :], start=True, stop=True)
                if use_bias:
                    nc.vector.tensor_scalar_add(h1p[:, 0:NTILE], h1p[:, 0:NTILE], b123[:, 0:1])
                    nc.vector.tensor_scalar_add(h1p[:, NTILE:], h1p[:, NTILE:], b123[:, 1:2])
                h1ps[t] = h1p

        def e_silu(live, src, dst):
            for (t, ts, isp, um) in live:
                hs = acts.tile([128, 2 * NTILE], BF16, tag="h")
                nc.scalar.activation(hs, src[t], Silu)
                dst[t] = hs

        def e_l23(live, w, src, dst, pool, boff):
            for i, (t, ts, isp, um) in enumerate(live):
                hp = pool[i % 2].tile([128, 2 * NTILE], F32, tag="h")
                for mt in range(2):
                    for kt in range(2):
                        nc.tensor.matmul(
                            hp[:, mt * NTILE:(mt + 1) * NTILE],
                            w[:, kt * 256 + mt * 128: kt * 256 + (mt + 1) * 128],
                            src[t][:, kt * NTILE:(kt + 1) * NTILE],
                            start=(kt == 0), stop=(kt == 1))
                if use_bias:
                    nc.vector.tensor_scalar_add(hp[:, 0:NTILE], hp[:, 0:NTILE], b123[:, boff:boff + 1])
                    nc.vector.tensor_scalar_add(hp[:, NTILE:], hp[:, NTILE:], b123[:, boff + 1:boff + 2])
                dst[t] = hp

        def e_l4_add(live):
            # L4 into a subregion of the (consumed) h3 psum tile, then the
            # DVE state update.  Masked evals read a host-precomputed 4-row
            # replicated mask slot — plain DVE mul, no broadcast matmul.
            for (t, ts, isp, um) in live:
                d = h3ps[t][0:4, 0:NTILE]
                nc.tensor.matmul(d, w4[:, 0:4], h3ss[t][:, 0:NTILE],
                                 start=True, stop=False)
                nc.tensor.matmul(d, w4[:, 4:8], h3ss[t][:, NTILE:2 * NTILE],
                                 start=False, stop=True)
            for (t, ts, isp, um) in live:
                d = h3ps[t][0:4, 0:NTILE]
                if use_bias:
                    nc.vector.tensor_scalar_add(d, d, b4r[:, 0:1])
                if not um:
                    nc.vector.tensor_add(xm[0:4, ts], xm[0:4, ts], d)
                else:
                    j = mask_slot[0]
                    mask_slot[0] += 1
                    dm = tmp.tile([4, NTILE], F32, tag="dm")
                    nc.vector.tensor_mul(dm, d, mkc[:, j * NTILE:(j + 1) * NTILE])
                    nc.vector.tensor_add(xm[0:4, ts], xm[0:4, ts], dm)

        # ---- two half-groups A/B per 4-tile group run half a round out of
        # phase: B's matmuls keep the PE busy while A's tail silus run, and
        # each half-group's L4+state-update is deferred into the opposite
        # phase so the PE never head-blocks on a just-issued silu.  The
        # flat masked-eval order stays A(0),B(0),A(1),B(1),... — identical
        # to _emit_order.
        def mklive(ts_, r):
            return [(t, bass.ds(t * NTILE, NTILE), *schedule[t][r])
                    for t in ts_ if r < len(schedule[t])]

        # prevB carries ACROSS groups and chunks: the final half-group's
        # deferred L4+add lands inside the next group's first window, so
        # group boundaries never head-block the PE on a tail silu.
        # (Output DMA stays at program end: emitting it mid-stream was
        # measured to serialize the pipeline, +220us.)
        prevB = []
        for (c0, c1) in chunks:
            for g0 in range(c0, c1, GROUP):
                gt = list(range(g0, min(g0 + GROUP, c1)))
                half = (len(gt) + 1) // 2
                A, B = gt[:half], gt[half:]
                R = max((len(schedule[t]) for t in gt), default=0)

                for r in range(R):
                    liveA = mklive(A, r)
                    liveB = mklive(B, r)
                    e_cast(liveA)                              # w1
                    e_l1(liveA)                                # w2
                    e_silu(liveA, h1ps, h1ss)                  # w3
                    e_l23(liveA, w2, h1ss, h2ps, ps12, 2)      # w4
                    e_l4_add(prevB)                            # w5
                    e_cast(liveB)
                    e_silu(liveA, h2ps, h2ss)                  # w6
                    e_l23(liveA, w3, h2ss, h3ps, ps3, 4)       # w7
                    e_silu(liveA, h3ps, h3ss)                  # w8
                    e_l1(liveB)                                # w9
                    e_silu(liveB, h1ps, h1ss)                  # w10
                    e_l23(liveB, w2, h1ss, h2ps, ps12, 2)      # w11
                    e_l4_add(liveA)                            # w12
                    e_silu(liveB, h2ps, h2ss)                  # w13
                    e_l23(liveB, w3, h2ss, h3ps, ps3, 4)       # w14
                    e_silu(liveB, h3ps, h3ss)                  # w15
                    prevB = liveB
        e_l4_add(prevB)                                        # program tail
        for (c0, c1) in chunks:
            nc.sync.dma_start(out=out_d[:, c0 * NTILE:c1 * NTILE],
                              in_=xm[0:4, c0 * NTILE:c1 * NTILE])

    nc.compile()
    return nc


def kernel(state_dz, z_frac, W1, b1, W2, b2, W3, b3, W4, b4):
    global LAST_EXEC_NS
    import os

    state_dz = np.ascontiguousarray(state_dz, dtype=np.float32)
    z_frac = np.ascontiguousarray(z_frac, dtype=np.float32)
    W1 = np.asarray(W1, np.float32); W2 = np.asarray(W2, np.float32)
    W3 = np.asarray(W3, np.float32); W4 = np.asarray(W4, np.float32)
    b1 = np.asarray(b1, np.float32); b2 = np.asarray(b2, np.float32)
    b3 = np.asarray(b3, np.float32); b4 = np.asarray(b4, np.float32)

    B = state_dz.shape[0]
    assert B % (NCORES * NTILE) == 0, f"B={B} must be divisible by {NCORES * NTILE}"
    n_core = B // NCORES
    tiles = n_core // NTILE

    # ---- host-side derived quantities (bitwise-identical fp32 ops vs jax)
    dz_sub = (state_dz[:, 5] / np.float32(8.0)).astype(np.float32)
    cont = (z_frac * np.float32(NSTEPS)).astype(np.float32)
    n_full = np.floor(cont).astype(np.float32)
    frac = (cont - n_full).astype(np.float32)
    dz_part = (frac * dz_sub).astype(np.float32)
    has_part = (frac > np.float32(1e-6)).astype(np.float32)
    n_int = np.minimum(n_full, NSTEPS).astype(np.int64)

    # ---- sort desc by n_full, deal round-robin to cores
    order = np.argsort(-n_int, kind="stable")
    perms = [order[c::NCORES] for c in range(NCORES)]

    # ---- build per-core xm arrays  [16, N_CORE]
    xms = []
    for c in range(NCORES):
        p = perms[c]
        xm = np.zeros((8, n_core), np.float32)
        xm[0:4] = state_dz[p, 0:4].T
        xm[R_QOP] = state_dz[p, 4]
        xm[R_DZSUB] = dz_sub[p]
        xm[R_DZPART] = dz_part[p]
        xms.append(xm)

    # ---- union schedule across cores (SPMD: one program for all cores)
    sched = []
    for t in range(tiles):
        sl = slice(t * NTILE, (t + 1) * NTILE)
        smax, smin = 0, NSTEPS
        anyp, allp = False, True
        for c in range(NCORES):
            nf = n_int[perms[c][sl]]
            smax = max(smax, int(nf.max()))
            smin = min(smin, int(nf.min()))
            hp = has_part[perms[c][sl]]
            anyp = anyp or bool(hp.any())
            allp = allp and bool(hp.all())
        evals = []
        steps = 0
        for s in range(min(smax, NSTEPS)):
            evals.append((False, smin <= s))
            steps += 1
        if anyp:
            evals.append((True, not allp))
        sched.append(tuple(evals))
    sched = tuple(sched)

    # masked evals in DEVICE EMISSION order; track each tile's step counter
    masked_evals = []
    step_no = [0] * tiles
    for t, (is_partial, use_mask) in _emit_order(sched, tiles):
        s = None if is_partial else step_no[t]
        if not is_partial:
            step_no[t] += 1
        if use_mask:
            masked_evals.append((t, s))

    # ---- packed mask rows (replicated to 4 partitions), one NTILE slot
    # per masked eval, per core
    nm = max(1, len(masked_evals))
    maskcats = [np.zeros((4, nm * NTILE), np.float32) for _ in range(NCORES)]
    for j, (t, s) in enumerate(masked_evals):
        sl = slice(t * NTILE, (t + 1) * NTILE)
        for c in range(NCORES):
            idx = perms[c][sl]
            row = has_part[idx] if s is None else (n_full[idx] > s).astype(np.float32)
            maskcats[c][:, j * NTILE:(j + 1) * NTILE] = row[None, :]

    use_bias = bool(np.any(b1) or np.any(b2) or np.any(b3) or np.any(b4))

    key = (sched, use_bias, n_core)
    if key not in _BUILD_CACHE:
        _BUILD_CACHE[key] = _build(sched, use_bias, n_core)
    nc = _BUILD_CACHE[key]

    # ---- weight tensors in lhsT layouts
    w1h = np.zeros((8, 512), np.float32)
    w1h[0:6, 0:256] = W1                      # full: state,qop,dz_sub
    w1h[0:5, 256:512] = W1[0:5]               # partial: dz slot zeroed,
    w1h[6, 256:512] = W1[5]                   # dz weight reads dz_partial row
    w2h = np.concatenate([W2[0:128], W2[128:256]], axis=1).astype(np.float32)
    w3h = np.concatenate([W3[0:128], W3[128:256]], axis=1).astype(np.float32)
    w4h = np.concatenate([W4[0:128], W4[128:256]], axis=1).astype(np.float32)

    in_map = {"w1": w1h, "w2": w2h, "w3": w3h, "w4": w4h}
    if use_bias:
        b123 = np.stack([b1[0:128], b1[128:256], b2[0:128], b2[128:256],
                         b3[0:128], b3[128:256]], axis=1).astype(np.float32)
        in_map["b123"] = b123
        in_map["b4r"] = b4.reshape(4, 1).astype(np.float32)

    in_maps = [{**in_map, "xm": xms[c], "maskcat": maskcats[c]}
               for c in range(NCORES)]

    trace = os.environ.get("BASSK_TRACE") == "1" and _install_ntff_hook()
    try:
        res = run_bass_kernel_spmd(nc, in_maps, list(range(NCORES)), trace=trace)
    except Exception:
        if not trace:
            raise
        res = run_bass_kernel_spmd(nc, in_maps, list(range(NCORES)), trace=False)
    LAST_EXEC_NS = res.exec_time_ns

    out = np.empty((B, 4), np.float32)
    for c in range(NCORES):
        out[perms[c], :] = res.results[c]["outT"].T
    return out
